# revision 13
# baseline (speedup 1.0000x reference)
"""Trainium2 Bass kernel for ObjectnessLoss (raw Bass, SPMD over 8 cores).

Math: reference computes  total = sum_b f_b^T D f_b / (B*N^2)  with
  f = minmax-normalized per-pixel |.|-channel-sum of input (B,64,64,3), N=4096,
  D[i,j] = Euclidean distance between pixel coords i,j on the 64x64 grid.

D[i,j] depends only on (dy, dx), so the bilinear form decomposes into 64
row-offset terms, each a 64x64 matmul against a precomputed distance table:
  S2(a) = a^T D a = sum_{d=0}^{63} w_d * sum_y A[y+d,:] @ R_d @ A[y,:]^T,
  R_d[x,x'] = sqrt((x-x')^2 + d^2), w_0=1, w_d=2 (symmetry).
Min-max normalization is deferred via f = alpha*a + beta:
  f^T D f = alpha^2 S2 + 2 alpha beta S1 + beta^2 S0,
  S1 = <a, D@1> (precomputed rowsum image), S0 = 1^T D 1 (host constant).

Sharding: data-parallel over batch; core b handles image b, returns
  [S2 partials x4, S1, -min(a), max(a)]; host combines 8x7 scalars.

On-core: image loaded transposed twice (upper half = A^T, lower half = A^T
shifted one column) -> At2z [128,128] zero-padded, so each K=128 matmul
covers offsets d and d+1: lhsT = At2z[:, d:d+64], rhs = [w_d R_d; w_{d+1}
R_{d+1}]. 32 matmuls -> 4 PSUM banks; fused multiply+accumulate-reduce
(scalar_tensor_tensor) of each bank against A; PE-transpose of the 7
per-partition partials + final free-dim reduces -> 7 scalars out.
"""

import os
import sys
from contextlib import ExitStack

import numpy as np

for _p in ("/opt/trn_rl_repo",):
    if _p not in sys.path and os.path.isdir(_p):
        sys.path.insert(0, _p)

import concourse.bass as bass
from concourse import mybir
from concourse import bass_utils

F32 = mybir.dt.float32
N_CORES = 8
B, H, W, C = 8, 64, 64, 3
N = H * W


def _build_tables():
    d = np.arange(64, dtype=np.float64)
    # R[d, x, x'] = sqrt((x-x')^2 + d^2)
    R = np.sqrt((d[None, :, None] - d[None, None, :]) ** 2 + d[:, None, None] ** 2)
    w = np.full(64, 2.0)
    w[0] = 1.0
    Rw = R * w[:, None, None]
    # rtab[p, k, n]: p<64 -> w[2k]*R[2k][p, n]; p>=64 -> w[2k+1]*R[2k+1][p-64, n]
    rtab = np.empty((128, 32, 64), dtype=np.float32)
    rtab[:64] = Rw[0::2].transpose(1, 0, 2)
    rtab[64:] = Rw[1::2].transpose(1, 0, 2)

    # dmat[y, x] = sum_j dist((y,x), j);  S0 = sum_ij dist
    yy, xx = np.meshgrid(np.arange(64), np.arange(64), indexing="ij")
    P = np.stack([yy.ravel(), xx.ravel()], 1).astype(np.float64)
    D = np.sqrt(((P[:, None, :] - P[None, :, :]) ** 2).sum(-1))
    dmat = D.sum(1).reshape(64, 64).astype(np.float32)
    S0 = float(D.sum())

    cmat = np.empty((2, 64, 64), dtype=np.float32)
    cmat[0] = dmat
    cmat[1] = np.eye(64, dtype=np.float32)
    return rtab, cmat, S0


_RTAB, _CMAT, _S0 = _build_tables()

MUL = mybir.AluOpType.mult
ADD = mybir.AluOpType.add
MAX = mybir.AluOpType.max
ABSMAX = mybir.AluOpType.abs_max
AXX = mybir.AxisListType.X


def _build_nc():
    nc = bass.Bass()
    x_h = nc.declare_dram_parameter("x", [H, W, C], F32, isOutput=False)
    rtab_h = nc.declare_dram_parameter("rtab", [128, 32, 64], F32, isOutput=False)
    cmat_h = nc.declare_dram_parameter("cmat", [2, 64, 64], F32, isOutput=False)
    out_h = nc.declare_dram_parameter("out", [7, 1], F32, isOutput=True)

    x0 = x_h[:, :, :]

    with ExitStack() as es:
        T128 = es.enter_context(nc.sbuf_tensor("T128", [128, 64, 3], F32))
        T64 = es.enter_context(nc.sbuf_tensor("T64", [64, 64, 3], F32))
        At2z = es.enter_context(nc.sbuf_tensor("At2z", [128, 128], F32))
        A = es.enter_context(nc.sbuf_tensor("A", [64, 64], F32))
        R2 = es.enter_context(nc.sbuf_tensor("R2", [128, 32, 64], F32))
        dmat = es.enter_context(nc.sbuf_tensor("dmat", [64, 64], F32))
        iden = es.enter_context(nc.sbuf_tensor("iden", [64, 64], F32))
        accs = es.enter_context(nc.sbuf_tensor("accs", [64, 8], F32))
        scr = es.enter_context(nc.sbuf_tensor("scr", [64, 8, 64], F32))
        negA = es.enter_context(nc.sbuf_tensor("negA", [64, 64], F32))
        outA = es.enter_context(nc.sbuf_tensor("outA", [5, 1], F32))
        outB = es.enter_context(nc.sbuf_tensor("outB", [2, 1], F32))
        PS = [es.enter_context(nc.psum_tensor(f"PS{j}", [64, 8, 64], F32))
              for j in range(4)]
        PSt1 = es.enter_context(nc.psum_tensor("PSt1", [5, 64], F32))
        PSt2 = es.enter_context(nc.psum_tensor("PSt2", [2, 64], F32))
        ta = es.enter_context(nc.semaphore("ta"))
        tb = es.enter_context(nc.semaphore("tb"))
        t64s = es.enter_context(nc.semaphore("t64s"))
        dms = es.enter_context(nc.semaphore("dms"))
        ids = es.enter_context(nc.semaphore("ids"))
        rs = [es.enter_context(nc.semaphore(f"r{j}")) for j in range(8)]
        msem = es.enter_context(nc.semaphore("msem"))
        vsem = es.enter_context(nc.semaphore("vsem"))
        osem = es.enter_context(nc.semaphore("osem"))
        block = es.enter_context(nc.Block())

        @block.sync
        def _(sync):
            # first rtab chunk + image loads first (critical path)
            sync.dma_start(out=R2[:, 0:4, :], in_=rtab_h[:, 0:4, :]).then_inc(rs[0], 16)
            sync.dma_start(
                out=T128[0:64, :, :],
                in_=bass.AP(tensor=x0.tensor, offset=x0.offset,
                            ap=[[3, 64], [192, 64], [1, 3]]),
            ).then_inc(ta, 16)
            sync.dma_start(
                out=T128[64:128, 0:63, :],
                in_=bass.AP(tensor=x0.tensor, offset=x0.offset + 192,
                            ap=[[3, 64], [192, 63], [1, 3]]),
            ).then_inc(tb, 16)
            for j in range(1, 8):
                sync.dma_start(out=R2[:, 4 * j:4 * j + 4, :],
                               in_=rtab_h[:, 4 * j:4 * j + 4, :]).then_inc(rs[j], 16)
            sync.dma_start(out=T64[:, :, :], in_=x0).then_inc(t64s, 16)
            sync.dma_start(out=dmat[:, :], in_=cmat_h[0]).then_inc(dms, 16)
            sync.dma_start(out=iden[:, :], in_=cmat_h[1]).then_inc(ids, 16)
            # stores at the very end
            sync.wait_ge(vsem, 3)
            sync.dma_start(out=out_h[0:5, :], in_=outA[:, :]).then_inc(osem, 16)
            sync.dma_start(out=out_h[5:7, :], in_=outB[:, :]).then_inc(osem, 16)
            sync.wait_ge(osem, 32)

        @block.vector
        def _(vector):
            vector.memset(At2z[:, :], 0.0)
            vector.wait_ge(ta, 16)
            nc.vector.tensor_reduce(At2z[0:64, 0:64], T128[0:64, :, :],
                                    axis=AXX, op=ADD, apply_absolute_value=True)
            vector.wait_ge(tb, 16)
            nc.vector.tensor_reduce(At2z[64:128, 0:63], T128[64:128, 0:63, :],
                                    axis=AXX, op=ADD,
                                    apply_absolute_value=True).then_inc(vsem, 1)
            # A in [y, x] layout
            vector.wait_ge(t64s, 16)
            nc.vector.tensor_reduce(A[:, :], T64[:, :, :], axis=AXX, op=ADD,
                                    apply_absolute_value=True)
            # S1 = <A, dmat>
            vector.wait_ge(dms, 16)
            nc.vector.scalar_tensor_tensor(
                out=scr[:, 0, :], in0=A[:, :], scalar=1.0, in1=dmat[:, :],
                op0=MUL, op1=MUL, accum_out=accs[:, 4:5])
            # -min, max partials
            nc.vector.tensor_scalar_mul(negA[:, :], A[:, :], -1.0)
            nc.vector.tensor_reduce(accs[:, 5:6], negA[:, :], axis=AXX, op=MAX)
            nc.vector.tensor_reduce(accs[:, 6:7], A[:, :], axis=AXX, op=MAX)
            # pairing: accs[:, j] = rowsum(PS_j * A-broadcast)
            a0 = A[:, :]
            a_b = bass.AP(tensor=a0.tensor, offset=a0.offset,
                          ap=[a0.ap[0], [0, 8], a0.ap[1]])
            for j in range(4):
                vector.wait_ge(msem, 8 * (j + 1))
                ins = nc.vector.scalar_tensor_tensor(
                    out=scr[:, :, :], in0=PS[j][:, :, :], scalar=1.0, in1=a_b,
                    op0=MUL, op1=MUL, accum_out=accs[:, j:j + 1])
                if j == 3:
                    ins.then_inc(vsem, 1)
            # final reduces after PE transposes
            vector.wait_ge(msem, 34)
            nc.vector.tensor_reduce(outA[:, 0:1], PSt1[:, :], axis=AXX, op=ADD)
            nc.vector.tensor_reduce(outB[:, 0:1], PSt2[:, :],
                                    axis=AXX, op=MAX).then_inc(vsem, 1)

        @block.tensor
        def _(tensor):
            tensor.wait_ge(vsem, 1)
            for j in range(4):
                for kk in range(8):
                    k = 8 * j + kk
                    if k % 4 == 0:
                        tensor.wait_ge(rs[k // 4], 16)
                    nc.tensor.matmul(
                        out=PS[j][:, kk, :],
                        lhsT=At2z[:, 2 * k:2 * k + 64],
                        rhs=R2[:, k, :],
                        start=True, stop=True,
                    ).then_inc(msem, 1)
            tensor.wait_ge(vsem, 2)
            tensor.wait_ge(ids, 16)
            nc.tensor.transpose(out=PSt1[:, :], in_=accs[:, 0:5],
                                identity=iden[:, :]).then_inc(msem, 1)
            nc.tensor.transpose(out=PSt2[:, :], in_=accs[:, 5:7],
                                identity=iden[:, :]).then_inc(msem, 1)

    return nc


_NC_CACHE = None


def _get_nc():
    global _NC_CACHE
    if _NC_CACHE is None:
        _NC_CACHE = _build_nc()
    return _NC_CACHE


def run_cores(x, trace=False):
    """x: (8,64,64,3) float32. Returns (per-core [7] arrays, BassKernelResults)."""
    nc = _get_nc()
    in_maps = [
        {"x": np.ascontiguousarray(x[b], dtype=np.float32),
         "rtab": _RTAB, "cmat": _CMAT}
        for b in range(N_CORES)
    ]
    res = bass_utils.run_bass_kernel_spmd(
        nc, in_maps, core_ids=list(range(N_CORES)), trace=trace
    )
    outs = [np.asarray(res.results[b]["out"]).reshape(7) for b in range(N_CORES)]
    return outs, res


def combine(outs):
    S2 = np.array([o[0:4].sum(dtype=np.float64) for o in outs])
    S1 = np.array([np.float64(o[4]) for o in outs])
    mn = min(-np.float64(o[5]) for o in outs)
    mx = max(np.float64(o[6]) for o in outs)
    alpha = 1.0 / (mx - mn)
    beta = -mn / (mx - mn)
    total = (alpha * alpha * S2.sum() + 2.0 * alpha * beta * S1.sum()
             + B * beta * beta * _S0)
    return np.array(total / (B * N * N), dtype=np.float32)


def kernel(**inputs):
    x = np.asarray(inputs["input"], dtype=np.float32)
    outs, _ = run_cores(x, trace=False)
    return combine(outs)


# revision 16
# speedup vs baseline: 1.4560x; 1.4560x over previous
"""Trainium2 Bass kernel for ObjectnessLoss (raw Bass, SPMD over 8 cores).

Math: reference computes  total = sum_b f_b^T D f_b / (B*N^2)  with
  f = minmax-normalized per-pixel |.|-channel-sum of input (B,64,64,3), N=4096,
  D[i,j] = Euclidean distance between pixel coords i,j on the 64x64 grid.

D[i,j] depends only on (dy, dx), so the bilinear form decomposes into 64
row-offset terms, each a 64x64 matmul against a precomputed distance table:
  S2(a) = a^T D a = sum_{d=0}^{63} w_d * sum_y A[y+d,:] @ R_d @ A[y,:]^T,
  R_d[x,x'] = sqrt((x-x')^2 + d^2), w_0=1, w_d=2 (symmetry).
Min-max normalization is deferred via f = alpha*a + beta:
  f^T D f = alpha^2 S2 + 2 alpha beta S1 + beta^2 S0,
  S1 = <a, D@1> (precomputed rowsum image), S0 = 1^T D 1 (host constant).

Sharding: data-parallel over batch; core b handles image b, returns
  [S2 partials x4, S1, -min(a), max(a)]; host combines 8x7 scalars.

On-core dataflow:
  x -> A[y,x] (abs + channel-sum via one tensor_reduce)
  A -> At2z [128,128]: upper = A^T, lower = A^T shifted one column -- built
    by two PE matmuls (A @ I and A @ Ishift into PSUM partitions 0-63 /
    64-127 via col-group tiling), then one PSUM->SBUF copy. The zero-padded
    pair layout lets each K=128 matmul cover offsets d and d+1.
  32 matmuls (fp32r): lhsT = At2z[:, d:d+64], rhs = [w_d R_d; w_{d+1}R_{d+1}]
    -> 4 PSUM banks; fused multiply+reduce (scalar_tensor_tensor) of each
    bank against A accumulates S2 partials; PE-transpose of the 7
    per-partition partials + final reduces -> 7 scalars out.
"""

import os
import sys
from contextlib import ExitStack

import numpy as np

for _p in ("/opt/trn_rl_repo",):
    if _p not in sys.path and os.path.isdir(_p):
        sys.path.insert(0, _p)

import concourse.bass as bass
from concourse import mybir
from concourse import bass_utils

F32 = mybir.dt.float32
F32R = mybir.dt.float32r
N_CORES = 8
B, H, W, C = 8, 64, 64, 3
N = H * W


def _build_tables():
    d = np.arange(64, dtype=np.float64)
    # R[d, x, x'] = sqrt((x-x')^2 + d^2)
    R = np.sqrt((d[None, :, None] - d[None, None, :]) ** 2 + d[:, None, None] ** 2)
    w = np.full(64, 2.0)
    w[0] = 1.0
    Rw = R * w[:, None, None]
    # rtab[p, k, n]: p<64 -> w[2k]*R[2k][p, n]; p>=64 -> w[2k+1]*R[2k+1][p-64, n]
    rtab = np.empty((128, 32, 64), dtype=np.float32)
    rtab[:64] = Rw[0::2].transpose(1, 0, 2)
    rtab[64:] = Rw[1::2].transpose(1, 0, 2)

    # dmat[y, x] = sum_j dist((y,x), j);  S0 = sum_ij dist
    yy, xx = np.meshgrid(np.arange(64), np.arange(64), indexing="ij")
    P = np.stack([yy.ravel(), xx.ravel()], 1).astype(np.float64)
    D = np.sqrt(((P[:, None, :] - P[None, :, :]) ** 2).sum(-1))
    dmat = D.sum(1).reshape(64, 64).astype(np.float32)
    S0 = float(D.sum())

    cmat = np.empty((3, 64, 64), dtype=np.float32)
    cmat[0] = dmat
    cmat[1] = np.eye(64, dtype=np.float32)
    cmat[2] = np.eye(64, k=-1, dtype=np.float32)  # Ishift[y, y'] = [y == y'+1]
    return rtab, cmat, S0


_RTAB, _CMAT, _S0 = _build_tables()

MUL = mybir.AluOpType.mult
ADD = mybir.AluOpType.add
MAX = mybir.AluOpType.max
AXX = mybir.AxisListType.X


def _build_nc():
    nc = bass.Bass()
    x_h = nc.declare_dram_parameter("x", [H, W, C], F32, isOutput=False)
    rtab_h = nc.declare_dram_parameter("rtab", [128, 32, 64], F32R, isOutput=False)
    cmat_h = nc.declare_dram_parameter("cmat", [3, 64, 64], F32, isOutput=False)
    out_h = nc.declare_dram_parameter("out", [7, 1], F32, isOutput=True)

    with ExitStack() as es:
        T64 = es.enter_context(nc.sbuf_tensor("T64", [64, 64, 3], F32))
        A = es.enter_context(nc.sbuf_tensor("A", [64, 64], F32))
        At2z = es.enter_context(nc.sbuf_tensor("At2z", [128, 128], F32R))
        R2 = es.enter_context(nc.sbuf_tensor("R2", [128, 32, 64], F32R))
        cm = es.enter_context(nc.sbuf_tensor("cm", [64, 3, 64], F32))
        accs = es.enter_context(nc.sbuf_tensor("accs", [64, 8], F32))
        scr = es.enter_context(nc.sbuf_tensor("scr", [64, 8, 64], F32))
        negA = es.enter_context(nc.sbuf_tensor("negA", [64, 64], F32))
        outA = es.enter_context(nc.sbuf_tensor("outA", [5, 1], F32))
        outB = es.enter_context(nc.sbuf_tensor("outB", [2, 1], F32))
        At2zP = es.enter_context(nc.psum_tensor("At2zP", [128, 64], F32))
        PS = [es.enter_context(nc.psum_tensor(f"PS{j}", [64, 8, 64], F32))
              for j in range(4)]
        PSt1 = es.enter_context(nc.psum_tensor("PSt1", [5, 64], F32))
        PSt2 = es.enter_context(nc.psum_tensor("PSt2", [2, 64], F32))
        t64s = es.enter_context(nc.semaphore("t64s"))
        csem = es.enter_context(nc.semaphore("csem"))
        rs = [es.enter_context(nc.semaphore(f"r{j}")) for j in range(8)]
        msem = es.enter_context(nc.semaphore("msem"))
        vsem = es.enter_context(nc.semaphore("vsem"))
        osem = es.enter_context(nc.semaphore("osem"))
        block = es.enter_context(nc.Block())

        dmat = cm[:, 0, :]
        iden = cm[:, 1, :]
        ishf = cm[:, 2, :]
        c0 = cmat_h[:, :, :]

        @block.sync
        def _(sync):
            # cmat as [64, 3, 64] (partition = dim1 of cmat)
            sync.dma_start(
                out=cm[:, :, :],
                in_=bass.AP(tensor=c0.tensor, offset=c0.offset,
                            ap=[[64, 64], [4096, 3], [1, 64]]),
            ).then_inc(csem, 16)
            for j in range(4):
                sync.dma_start(out=R2[:, 4 * j:4 * j + 4, :],
                               in_=rtab_h[:, 4 * j:4 * j + 4, :]).then_inc(rs[j], 16)
            sync.wait_ge(vsem, 4)
            sync.dma_start(out=out_h[0:5, :], in_=outA[:, :]).then_inc(osem, 16)
            sync.dma_start(out=out_h[5:7, :], in_=outB[:, :]).then_inc(osem, 16)
            sync.wait_ge(osem, 32)

        @block.scalar
        def _(scalar):
            scalar.dma_start(out=T64[:, :, :], in_=x_h[:, :, :]).then_inc(t64s, 16)
            for j in range(4, 8):
                scalar.dma_start(out=R2[:, 4 * j:4 * j + 4, :],
                                 in_=rtab_h[:, 4 * j:4 * j + 4, :]).then_inc(rs[j], 16)

        @block.vector
        def _(vector):
            vector.memset(At2z[:, :].bitcast(F32), 0.0)
            # A[y, x] = sum_c |x[y, x, c]|
            vector.wait_ge(t64s, 16)
            nc.vector.tensor_reduce(A[:, :], T64[:, :, :], axis=AXX, op=ADD,
                                    apply_absolute_value=True).then_inc(vsem, 1)
            # At2z <- PSUM transpose pair
            vector.wait_ge(msem, 2)
            nc.vector.tensor_copy(out=At2z[:, 0:64],
                                  in_=At2zP[:, :]).then_inc(vsem, 1)
            # S1 = <A, dmat>
            vector.wait_ge(csem, 16)
            nc.vector.scalar_tensor_tensor(
                out=scr[:, 0, :], in0=A[:, :], scalar=1.0, in1=dmat,
                op0=MUL, op1=MUL, accum_out=accs[:, 4:5])
            # -min, max partials
            nc.vector.tensor_scalar_mul(negA[:, :], A[:, :], -1.0)
            nc.vector.tensor_reduce(accs[:, 5:6], negA[:, :], axis=AXX, op=MAX)
            nc.vector.tensor_reduce(accs[:, 6:7], A[:, :], axis=AXX, op=MAX)
            # pairing: accs[:, j] = rowsum(PS_j * A-broadcast)
            a0 = A[:, :]
            a_b = bass.AP(tensor=a0.tensor, offset=a0.offset,
                          ap=[a0.ap[0], [0, 8], a0.ap[1]])
            for j in range(4):
                vector.wait_ge(msem, 2 + 8 * (j + 1))
                ins = nc.vector.scalar_tensor_tensor(
                    out=scr[:, :, :], in0=PS[j][:, :, :], scalar=1.0, in1=a_b,
                    op0=MUL, op1=MUL, accum_out=accs[:, j:j + 1])
                if j == 3:
                    ins.then_inc(vsem, 1)
            # final reduces after PE transposes
            vector.wait_ge(msem, 36)
            nc.vector.tensor_reduce(outA[:, 0:1], PSt1[:, :], axis=AXX, op=ADD)
            nc.vector.tensor_reduce(outB[:, 0:1], PSt2[:, :],
                                    axis=AXX, op=MAX).then_inc(vsem, 1)

        @block.tensor
        def _(tensor):
            tensor.wait_ge(vsem, 1)
            tensor.wait_ge(csem, 16)
            # At2zP[0:64]  = A^T           (x partitions 0-63)
            # At2zP[64:]   = (A shifted)^T (x partitions 64-127, col-group 2)
            nc.tensor.matmul(out=At2zP[0:64, :], lhsT=A[:, :], rhs=iden,
                             start=True, stop=True).then_inc(msem, 1)
            nc.tensor.matmul(out=At2zP[64:128, :], lhsT=A[:, :], rhs=ishf,
                             start=True, stop=True,
                             tile_position=(0, 64)).then_inc(msem, 1)
            tensor.wait_ge(vsem, 2)
            for j in range(4):
                for kk in range(8):
                    k = 8 * j + kk
                    if k % 4 == 0:
                        tensor.wait_ge(rs[k // 4], 16)
                    nc.tensor.matmul(
                        out=PS[j][:, kk, :],
                        lhsT=At2z[:, 2 * k:2 * k + 64],
                        rhs=R2[:, k, :],
                        start=True, stop=True,
                    ).then_inc(msem, 1)
            tensor.wait_ge(vsem, 3)
            nc.tensor.transpose(out=PSt1[:, :], in_=accs[:, 0:5],
                                identity=iden).then_inc(msem, 1)
            nc.tensor.transpose(out=PSt2[:, :], in_=accs[:, 5:7],
                                identity=iden).then_inc(msem, 1)

    return nc


_NC_CACHE = None


def _get_nc():
    global _NC_CACHE
    if _NC_CACHE is None:
        _NC_CACHE = _build_nc()
    return _NC_CACHE


def run_cores(x, trace=False):
    """x: (8,64,64,3) float32. Returns (per-core [7] arrays, BassKernelResults)."""
    nc = _get_nc()
    in_maps = [
        {"x": np.ascontiguousarray(x[b], dtype=np.float32),
         "rtab": _RTAB, "cmat": _CMAT}
        for b in range(N_CORES)
    ]
    res = bass_utils.run_bass_kernel_spmd(
        nc, in_maps, core_ids=list(range(N_CORES)), trace=trace
    )
    outs = [np.asarray(res.results[b]["out"]).reshape(7) for b in range(N_CORES)]
    return outs, res


def combine(outs):
    S2 = np.array([o[0:4].sum(dtype=np.float64) for o in outs])
    S1 = np.array([np.float64(o[4]) for o in outs])
    mn = min(-np.float64(o[5]) for o in outs)
    mx = max(np.float64(o[6]) for o in outs)
    alpha = 1.0 / (mx - mn)
    beta = -mn / (mx - mn)
    total = (alpha * alpha * S2.sum() + 2.0 * alpha * beta * S1.sum()
             + B * beta * beta * _S0)
    return np.array(total / (B * N * N), dtype=np.float32)


def kernel(**inputs):
    x = np.asarray(inputs["input"], dtype=np.float32)
    outs, _ = run_cores(x, trace=False)
    return combine(outs)
